# revision 2
# baseline (speedup 1.0000x reference)
"""Causal self-attention (B=2, T=2048, D=1024, H=16, Dh=64) on 8 TRN2 cores.

Sharding: core c -> batch b = c//4 (data parallel), head group g = c%4
(tensor parallel, 4 heads = 256 dims per group). Each core computes a
full-shape [T, D] partial of the output projection for its (b, g); the
host sums the 4 head-group partials per batch.

Per-core kernel (all matmuls fp32r = full-rate ~16-bit-mantissa):
  phase 1: qT/kT [256, T] and v [T, 256] projections from xT [1024, T]
  phase 2: per head: S.T = kT_h^T-tile @ qT_h (causal extent), exp on
           ScalarE (scale=1/8), diag-tile mask on VectorE, then
           O.T[65, T] += v~_h[tk,65] (stationary, ones col -> row sums)
           streamed against P.T; normalize via reciprocal + gpsimd
           partition-broadcast.
  phase 3: out[t, :] = y.T-tiles (stationary) @ woT.
"""

import numpy as np
from contextlib import ExitStack

import concourse.bass as bass
import concourse.tile as tile
from concourse import bacc, mybir
from concourse.bass_utils import run_bass_kernel_spmd

F32 = mybir.dt.float32
F32R = mybir.dt.float32r

B, T, D = 2, 2048, 1024
H_TOT, DH = 16, 64
HL = 4                # local heads per core
DG = HL * DH          # 256 local head dims
NT = T // 128         # 16 t-tiles
NCH = T // 512        # 4 t-chunks
CT = D // 128         # 8 c-tiles
PIECE = 1024          # S.T piece size (2 PSUM banks)

_CACHE = {}


def build():
    nc = bacc.Bacc("TRN2", target_bir_lowering=False, debug=False, num_devices=8)
    xT_d = nc.dram_tensor("xT", [D, T], F32R, kind="ExternalInput").ap()
    wq_d = nc.dram_tensor("wq", [D, DG], F32R, kind="ExternalInput").ap()
    wk_d = nc.dram_tensor("wk", [D, DG], F32R, kind="ExternalInput").ap()
    wv_d = nc.dram_tensor("wv", [D, DG], F32R, kind="ExternalInput").ap()
    wo_d = nc.dram_tensor("wo", [DG, D], F32R, kind="ExternalInput").ap()
    mask_d = nc.dram_tensor("mask", [128, 128], F32R, kind="ExternalInput").ap()
    ones_d = nc.dram_tensor("ones", [128, NT * HL], F32R, kind="ExternalInput").ap()
    out_d = nc.dram_tensor("out", [T, D], F32, kind="ExternalOutput").ap()

    with tile.TileContext(nc) as tc:
        with ExitStack() as ctx:
            cons = ctx.enter_context(tc.tile_pool(name="cons", bufs=1))
            xp = ctx.enter_context(tc.tile_pool(name="xp", bufs=2))
            cp = ctx.enter_context(tc.tile_pool(name="cp", bufs=3))
            pp = ctx.enter_context(tc.tile_pool(name="pp", bufs=3))
            outp = ctx.enter_context(tc.tile_pool(name="outp", bufs=3))

            # ---- constants / weights ----
            wq_sb = cons.tile([128, CT, DG], F32R)
            wk_sb = cons.tile([128, CT, DG], F32R)
            wv_sb = cons.tile([128, CT, DG], F32R)
            nc.sync.dma_start(wq_sb[:], wq_d.rearrange("(ct p) j -> p ct j", p=128))
            nc.sync.dma_start(wk_sb[:], wk_d.rearrange("(ct p) j -> p ct j", p=128))
            nc.sync.dma_start(wv_sb[:], wv_d.rearrange("(ct p) j -> p ct j", p=128))
            wo_sb = cons.tile([128, 2, D], F32R)
            nc.sync.dma_start(wo_sb[:], wo_d.rearrange("(g p) o -> p g o", p=128))
            mask_sb = cons.tile([128, 128], F32R)
            nc.sync.dma_start(mask_sb[:], mask_d[:])

            qsb = cons.tile([128, 2, T], F32R)
            ksb = cons.tile([128, 2, T], F32R)
            v_sb = cons.tile([128, NT, HL, DH + 1], F32R)
            nc.sync.dma_start(
                v_sb[:, :, :, DH],
                ones_d.rearrange("p (t h) -> p t h", h=HL),
            )
            y_sb = cons.tile([128, 2, T], F32R)

            # ---- phase 1: projections ----
            with tc.tile_pool(name="ps1", bufs=4, space="PSUM") as ps1:
                for n in range(NCH):
                    x_sb = xp.tile([128, CT, 512], F32R)
                    nc.sync.dma_start(
                        x_sb[:],
                        xT_d[:, 512 * n : 512 * (n + 1)].rearrange(
                            "(ct p) t -> p ct t", p=128
                        ),
                    )
                    for w_sb, dst in ((wq_sb, qsb), (wk_sb, ksb)):
                        for j2 in range(2):
                            pq = ps1.tile([128, 512], F32, tag="pq")
                            for ct in range(CT):
                                nc.tensor.matmul(
                                    pq[:],
                                    w_sb[:, ct, 128 * j2 : 128 * (j2 + 1)],
                                    x_sb[:, ct, :],
                                    start=(ct == 0),
                                    stop=(ct == CT - 1),
                                )
                            nc.vector.tensor_copy(
                                dst[:, j2, 512 * n : 512 * (n + 1)], pq[:]
                            )
                    for i in range(4):
                        ti = 4 * n + i
                        pv = ps1.tile([128, DG], F32, tag="pv")
                        for ct in range(CT):
                            nc.tensor.matmul(
                                pv[:],
                                x_sb[:, ct, 128 * i : 128 * (i + 1)],
                                wv_sb[:, ct, :],
                                start=(ct == 0),
                                stop=(ct == CT - 1),
                            )
                        nc.vector.tensor_copy(
                            v_sb[:, ti, :, 0:DH],
                            pv[:].rearrange("p (h d) -> p h d", h=HL),
                        )

            # ---- phase 2: attention per head ----
            with (
                tc.tile_pool(name="spool", bufs=2, space="PSUM") as spool,
                tc.tile_pool(name="opool", bufs=1, space="PSUM") as opool,
            ):
                for h in range(HL):
                    hp = 64 * (h % 2)       # partition offset
                    h2 = h // 2             # free tile index
                    oT = opool.tile([DH + 1, T], F32, name=f"oT_{h}", tag="oT")
                    for j in range(NT):
                        ext = T - 128 * j
                        for r0 in range(0, ext, PIECE):
                            r1 = min(r0 + PIECE, ext)
                            sT = spool.tile([128, PIECE], F32, tag="sT")
                            for c0 in range(r0, r1, 512):
                                c1 = min(c0 + 512, r1)
                                nc.tensor.matmul(
                                    sT[:, c0 - r0 : c1 - r0],
                                    ksb[hp : hp + DH, h2, 128 * j : 128 * (j + 1)],
                                    qsb[hp : hp + DH, h2, 128 * j + c0 : 128 * j + c1],
                                    start=True,
                                    stop=True,
                                )
                            pT = pp.tile([128, PIECE], F32R, tag="pT")
                            nc.scalar.activation(
                                pT[:, : r1 - r0],
                                sT[:, : r1 - r0],
                                mybir.ActivationFunctionType.Exp,
                                scale=0.125,
                            )
                            if r0 == 0:
                                nc.vector.tensor_mul(
                                    pT[:, 0:128], pT[:, 0:128], mask_sb[:]
                                )
                            # PV, split at absolute 512 (PSUM bank) boundaries
                            a0 = 128 * j + r0
                            a_end = 128 * j + r1
                            while a0 < a_end:
                                a1 = min((a0 // 512 + 1) * 512, a_end)
                                jstop = min(NT - 1, 4 * (a0 // 512) + 3)
                                nc.tensor.matmul(
                                    oT[:, a0:a1],
                                    v_sb[:, j, h, :],
                                    pT[:, a0 - 128 * j - r0 : a1 - 128 * j - r0],
                                    start=(j == 0),
                                    stop=(j == jstop),
                                    skip_group_check=True,
                                )
                                a0 = a1
                    # normalize: y.T rows = O.T[0:64] * (1/l) broadcast
                    r_sb = cp.tile([1, T], F32, tag="r")
                    nc.vector.reciprocal(r_sb[:], oT[DH : DH + 1, :])
                    rb_sb = cp.tile([DH, T], F32, tag="rb")
                    nc.gpsimd.partition_broadcast(rb_sb[:], r_sb[:])
                    nc.vector.tensor_mul(
                        y_sb[hp : hp + DH, h2, :], oT[0:DH, :], rb_sb[:]
                    )

            # ---- phase 3: output projection ----
            with tc.tile_pool(name="ps3", bufs=4, space="PSUM") as ps3:
                for i in range(NT):
                    for oc in range(2):
                        po = ps3.tile([128, 512], F32, tag="po")
                        for g2 in range(2):
                            nc.tensor.matmul(
                                po[:],
                                y_sb[:, g2, 128 * i : 128 * (i + 1)],
                                wo_sb[:, g2, 512 * oc : 512 * (oc + 1)],
                                start=(g2 == 0),
                                stop=(g2 == 1),
                            )
                        o_sb = outp.tile([128, 512], F32, tag="o")
                        nc.vector.tensor_copy(o_sb[:], po[:])
                        nc.sync.dma_start(
                            out_d[
                                128 * i : 128 * (i + 1), 512 * oc : 512 * (oc + 1)
                            ],
                            o_sb[:],
                        )
    nc.compile()
    return nc


def make_in_maps(x, Wq, Wk, Wv, Wo):
    mask = np.triu(np.ones((128, 128), dtype=np.float32))  # [tk, tq] valid tk<=tq
    ones = np.ones((128, NT * HL), dtype=np.float32)
    in_maps = []
    for c in range(8):
        b, g = c // 4, c % 4
        rows = slice(DG * g, DG * (g + 1))
        in_maps.append(
            {
                "xT": np.ascontiguousarray(x[b].T),
                "wq": np.ascontiguousarray(Wq[rows].T),
                "wk": np.ascontiguousarray(Wk[rows].T),
                "wv": np.ascontiguousarray(Wv[rows].T),
                "wo": np.ascontiguousarray(Wo[:, rows].T),
                "mask": mask,
                "ones": ones,
            }
        )
    return in_maps


def _run(x, Wq, Wk, Wv, Wo, trace=False):
    if "nc" not in _CACHE:
        _CACHE["nc"] = build()
    nc = _CACHE["nc"]
    in_maps = make_in_maps(x, Wq, Wk, Wv, Wo)
    res = run_bass_kernel_spmd(nc, in_maps, core_ids=list(range(8)), trace=trace)
    out = np.zeros((B, T, D), dtype=np.float32)
    for c in range(8):
        out[c // 4] += res.results[c]["out"]
    return out, res


def kernel(x, Wq, Wk, Wv, Wo):
    out, _ = _run(
        np.asarray(x, dtype=np.float32),
        np.asarray(Wq, dtype=np.float32),
        np.asarray(Wk, dtype=np.float32),
        np.asarray(Wv, dtype=np.float32),
        np.asarray(Wo, dtype=np.float32),
    )
    return out
